# revision 10
# baseline (speedup 1.0000x reference)
"""Trainium2 Bass kernel for CapsuleLayer (dynamic routing) on 8 NeuronCores.

Problem: x[32,1152,64], W[1152,32,64,64], bias[1,1152,32,1] (zeros) ->
         out[32,32,64]
  inputs_hat = einsum('bip,icpq->bicq', x, W)
  3 rounds of routing (softmax over capsule axis, squash, agreement update).

Sharding: input-capsule axis i=1152 split over 8 cores (144 each).
W is read exactly once per core (37.7 MB bf16); only the [32,32,64]
pre-squash sum is AllReduced per routing round (split in two so the first
half overlaps the tail of the round's compute).

v2 layout (vs the HBM-ih baseline):
  - Phase 1 packs FOUR i's per PSUM block: lhsT [128,128] with the even
    i-pair in cols 0:64 and the odd pair in cols 64:128 (zeros elsewhere),
    two matmuls per [128,512] block (start/stop accumulate). Output
    partitions = (islot, b) = 32*islot + b.
  - ih lives in SBUF for the whole kernel ([128, 36*2048] bf16, 144KB per
    partition): no ih HBM write + no 2x reload in the routing rounds.
  - s0 (= sum_i ih / 32, the uniform round-0 weighted sum) via selector
    matmuls (sel/32) over the finished SBUF ih tiles: 4 matmuls of N=512
    per quad, half the PE cost of the baseline's extra-lhsT-column scheme.
  - W tiles stream on two DMA queues (sync + gpsimd issue alternately);
    PSUM evacuation (fp32->bf16) is split Scalar/Vector/Pool.
  - Routing sweeps: scalar_tensor_tensor for the two big [128,2048] muls
    (4x DVE mode), in-place pair-add tree + [128,(32c),16q] reduce for the
    logit dot products, softmax 1/Z folded into the exp-broadcast
    activation's per-partition scale so the selector lhsT is constant
    (one LDWEIGHTS per round), agreement-weighted ih (wtt) computed
    in-place over the exp-broadcast tile.
"""

import os
import sys

import numpy as np

for _p in (
    "/opt/trn_rl_repo",
    "/root/.axon_site",
    "/root/.axon_site/_ro/trn_rl_repo",
    "/root/.axon_site/_ro/pypackages",
):
    if os.path.isdir(_p) and _p not in sys.path:
        sys.path.append(_p)

import ml_dtypes
import concourse.bacc as bacc
import concourse.mybir as mybir
import concourse.tile as tile
from concourse.bass_utils import run_bass_kernel_spmd

F32 = mybir.dt.float32
BF16 = mybir.dt.bfloat16
AF = mybir.ActivationFunctionType
AX = mybir.AxisListType
ALU = mybir.AluOpType
BF = ml_dtypes.bfloat16

B, I, P, C, Q = 32, 1152, 64, 32, 64
N_CORES = 8
IL = I // N_CORES          # 144 input capsules per core
NQUAD = IL // 4            # 36 quads (4 i's x 32 b = 128 partitions)
CQ = C * Q                 # 2048
NUM_ROUTING = 3
SPLIT_QUAD = 26            # s0 partial AllReduced after this many quads
SPLIT_SWEEP = 30           # routing partial AllReduced after this many sweeps

CONFIG = {
    "trace": False,           # profile the run (exec_time_ns); needs ntff hook
    "trace_cores": None,      # None -> core 0 only
}

_compiled = None


def _build_kernel():
    """Build + compile the SPMD Bass module (identical program on 8 cores)."""
    nc = bacc.Bacc("TRN2", target_bir_lowering=False, debug=False,
                   num_devices=N_CORES)

    # lhsT[p, q*256 + (0:128)] = even-pair weights, (128:256) = odd-pair
    lall_d = nc.dram_tensor("lhsT", [128, NQUAD * 256], BF16,
                            kind="ExternalInput")
    # w2[q, 0] = [W(4q);W(4q+1)] stacked on p, [q, 1] = [W(4q+2);W(4q+3)]
    w_d = nc.dram_tensor("w_rhs", [NQUAD, 2, 128, CQ], BF16,
                         kind="ExternalInput")
    sel1_d = nc.dram_tensor("sel1", [128, 32], BF16, kind="ExternalInput")
    sel32_d = nc.dram_tensor("sel32", [128, 32], BF16, kind="ExternalInput")
    out_d = nc.dram_tensor("out", [B, CQ], F32, kind="ExternalOutput")

    rgroups = [list(range(N_CORES))]

    with tile.TileContext(nc) as tc:
        with (
            tc.tile_pool(name="ihp", bufs=1) as ih_pool,
            tc.tile_pool(name="lq", bufs=6) as lq_pool,
            tc.tile_pool(name="w", bufs=4) as w_pool,
            tc.tile_pool(name="pr", bufs=2) as pr_pool,
            tc.tile_pool(name="e4", bufs=2) as e4_pool,
            tc.tile_pool(name="v4", bufs=1) as v4_pool,
            tc.tile_pool(name="small", bufs=3) as small_pool,
            tc.tile_pool(name="acc", bufs=1) as acc_pool,
            tc.tile_pool(name="sv", bufs=1) as sv_pool,
            tc.tile_pool(name="psA", bufs=4, space="PSUM") as psA,
            tc.tile_pool(name="psB", bufs=1, space="PSUM") as psB,
            tc.tile_pool(name="dram", bufs=2, space="DRAM") as dram_pool,
        ):
            sel1_t = small_pool.tile([128, 32], BF16, tag="sel1")
            nc.sync.dma_start(sel1_t[:], sel1_d[:])
            sel32_t = small_pool.tile([128, 32], BF16, tag="sel32")
            nc.sync.dma_start(sel32_t[:], sel32_d[:])

            b_acc = acc_pool.tile([128, NQUAD * 32], F32, tag="bacc")
            nc.vector.memset(b_acc[:], 0.0)

            # ih for the whole core, SBUF-resident: [128=(islot,b), 36*2048]
            ihb = ih_pool.tile([128, NQUAD * CQ], BF16, tag="ihb")

            # Warm-up AllReduce: the first collective pays a large one-time
            # staging/rendezvous cost; burn it here, hidden under phase 1.
            wu_sb = small_pool.tile([32, 16], F32, tag="wu")
            nc.vector.memset(wu_sb[:], 0.0)
            wu_in = dram_pool.tile([32, 16], F32, tag="wu_in")
            wu_out = dram_pool.tile([32, 16], F32, tag="wu_out")
            nc.gpsimd.dma_start(wu_in[:], wu_sb[:])
            nc.gpsimd.collective_compute(
                "AllReduce", ALU.add,
                ins=[wu_in[:].opt()], outs=[wu_out[:].opt()],
                replica_groups=rgroups,
            )

            def flush_and_allreduce(s_ps, tag):
                """PSUM partial -> SBUF -> DRAM -> AllReduce. Returns the
                collective's DRAM output tile (read back as [128, 512])."""
                f_sb = sv_pool.tile([32, CQ], F32, tag="f_sb")
                nc.scalar.copy(f_sb[:], s_ps[:])
                a_in = dram_pool.tile([32, CQ], F32, tag="ar_in")
                a_out = dram_pool.tile([32, CQ], F32, tag="ar_out")
                nc.gpsimd.dma_start(a_in[:], f_sb[:])
                nc.gpsimd.collective_compute(
                    "AllReduce", ALU.add,
                    ins=[a_in[:].opt()], outs=[a_out[:].opt()],
                    replica_groups=rgroups,
                )
                red = sv_pool.tile([128, CQ // 4], F32, tag=tag)
                nc.gpsimd.dma_start(red[:], a_out[:])
                return red

            # ---------------- Phase 1: ih = x @ W, s0 = sum_i ih / 32 -------
            def s0_matmul(q, s_ps, first, last):
                for blk in range(4):
                    sl = slice(512 * blk, 512 * (blk + 1))
                    nc.tensor.matmul(
                        s_ps[:, sl], sel32_t[:],
                        ihb[:, q * CQ + 512 * blk:q * CQ + 512 * (blk + 1)],
                        start=first, stop=last)

            s_ps = psB.tile([32, CQ], F32, tag="sacc")
            ar_handles = []
            evac_engines = [
                lambda o, i: nc.scalar.copy(o, i),
                lambda o, i: nc.vector.tensor_copy(o, i),
                lambda o, i: nc.scalar.copy(o, i),
                lambda o, i: nc.vector.tensor_copy(o, i),
            ]
            for q in range(NQUAD):
                lq = lq_pool.tile([128, 256], BF16, tag="lq")
                nc.sync.dma_start(lq[:], lall_d[:, 256 * q:256 * (q + 1)])
                we = w_pool.tile([128, CQ], BF16, tag="w")
                nc.sync.dma_start(we[:], w_d[q, 0])
                wo = w_pool.tile([128, CQ], BF16, tag="w")
                nc.gpsimd.dma_start(wo[:], w_d[q, 1])
                pss = []
                for blk in range(4):
                    sl = slice(512 * blk, 512 * (blk + 1))
                    ps = psA.tile([128, 512], F32)
                    pss.append(ps)
                    nc.tensor.matmul(ps[:], lq[:, 0:128], we[:, sl],
                                     start=True, stop=False)
                # s0 for the previous quad between the even/odd groups so the
                # PE never waits on the current quad's evacuation
                if q > 0:
                    qc = q - 1
                    s0_matmul(qc, s_ps,
                              first=(qc == 0 or qc == SPLIT_QUAD),
                              last=(qc == SPLIT_QUAD - 1 or qc == NQUAD - 1))
                    if qc == SPLIT_QUAD - 1:
                        ar_handles.append(flush_and_allreduce(s_ps, "ra"))
                        s_ps = psB.tile([32, CQ], F32, tag="sacc")
                for blk in range(4):
                    sl = slice(512 * blk, 512 * (blk + 1))
                    nc.tensor.matmul(pss[blk][:], lq[:, 128:256], wo[:, sl],
                                     start=False, stop=True)
                    evac_engines[blk](
                        ihb[:, q * CQ + 512 * blk:q * CQ + 512 * (blk + 1)],
                        pss[blk][:])
            qc = NQUAD - 1
            s0_matmul(qc, s_ps, first=False, last=True)
            ar_handles.append(flush_and_allreduce(s_ps, "rb"))

            # ---------------- Routing rounds -------------------------------
            # Post-AllReduce S layout: [128, 512], partition p = 4*b + k
            # (k = c-octet), free = (8c, 64q). Squash runs on all 128 lanes.
            C8 = C // 4     # 8 capsules per partition
            for r in range(1, NUM_ROUTING + 1):
                pa, pb = ar_handles
                S_sb = sv_pool.tile([128, CQ // 4], F32, tag="S_sb")
                nc.vector.tensor_add(S_sb[:], pa[:], pb[:])

                # squash: v = S * sqrt(sq)/(1+sq),  sq = sum_q S^2
                S3 = S_sb[:].rearrange("b (c q) -> b c q", q=Q)
                sq = small_pool.tile([128, C8], F32, tag="sq")
                sqr = sv_pool.tile([128, CQ // 4], F32, tag="sqr")
                nc.vector.tensor_mul(sqr[:], S_sb[:], S_sb[:])
                nc.vector.reduce_sum(
                    sq[:], sqr[:].rearrange("b (c q) -> b c q", q=Q),
                    axis=AX.X)
                rt = small_pool.tile([128, C8], F32, tag="rt")
                nc.scalar.sqrt(rt[:], sq[:])
                onep = small_pool.tile([128, C8], F32, tag="onep")
                nc.vector.tensor_scalar_add(onep[:], sq[:], 1.0)
                rden = small_pool.tile([128, C8], F32, tag="rden")
                nc.vector.reciprocal(rden[:], onep[:])
                scale = small_pool.tile([128, C8], F32, tag="scale")
                nc.vector.tensor_mul(scale[:], rt[:], rden[:])
                scale_b = scale[:].unsqueeze(-1).broadcast_to((128, C8, Q))

                if r == NUM_ROUTING:
                    # v overwrites S_sb in place (fp32 output)
                    nc.vector.tensor_mul(S3, S3, scale_b)
                    nc.sync.dma_start(out_d[:], S_sb[:])
                    break

                # v at bf16, then broadcast to the 4 i-slots: v_c's
                # (b, octet)-major partition stream == b-major (c,q) rows,
                # so each v4 slot-group is a straight SBUF->SBUF copy.
                v_c = sv_pool.tile([128, CQ // 4], BF16, tag="v_c")
                nc.vector.tensor_mul(
                    v_c[:].rearrange("b (c q) -> b c q", q=Q), S3, scale_b)
                v4 = v4_pool.tile([128, CQ], BF16, tag="v4")
                for g in range(4):
                    eng = nc.gpsimd if g % 2 == 0 else nc.scalar
                    eng.dma_start(v4[32 * g:32 * (g + 1), :], v_c[:])

                ar_handles = []
                s_ps = psB.tile([32, CQ], F32, tag="sacc")
                for s in range(NQUAD):
                    if s == SPLIT_SWEEP:
                        ar_handles.append(flush_and_allreduce(s_ps, "ra"))
                        s_ps = psB.tile([32, CQ], F32, tag="sacc")
                    first, last_s = (s == 0 or s == SPLIT_SWEEP), \
                        (s == SPLIT_SWEEP - 1 or s == NQUAD - 1)
                    it = ihb[:, s * CQ:(s + 1) * CQ]
                    # logits: dlog[(i,b), c] = sum_q ih*v, via 4x-mode STT
                    # product + two in-place pair-adds + a [*,(32c),16q] reduce
                    pr = pr_pool.tile([128, CQ], BF16, tag="pr")
                    nc.vector.scalar_tensor_tensor(
                        pr[:], it, 1.0, v4[:], ALU.bypass, ALU.mult)
                    pr3 = pr[:].rearrange("p (c q) -> p c q", q=Q)
                    nc.vector.scalar_tensor_tensor(
                        pr3[:, :, 0:32], pr3[:, :, 0:32], 1.0,
                        pr3[:, :, 32:64], ALU.bypass, ALU.add)
                    nc.vector.scalar_tensor_tensor(
                        pr3[:, :, 0:16], pr3[:, :, 0:16], 1.0,
                        pr3[:, :, 16:32], ALU.bypass, ALU.add)
                    dlog = small_pool.tile([128, C], F32, tag="dlog")
                    nc.vector.reduce_sum(dlog[:], pr3[:, :, 0:16], axis=AX.X)
                    bsl = b_acc[:, 32 * s:32 * (s + 1)]
                    nc.vector.tensor_add(bsl, bsl, dlog[:])
                    # softmax over c: exp on ACT (Z via accumulator); 1/Z is
                    # folded into the exp->(c,q) broadcast's input scale.
                    e = small_pool.tile([128, C], BF16, tag="e")
                    z = small_pool.tile([128, 1], F32, tag="z")
                    nc.scalar.activation(e[:], bsl, AF.Exp, accum_out=z[:])
                    rz = small_pool.tile([128, 1], F32, tag="rz")
                    nc.vector.reciprocal(rz[:], z[:])
                    e4 = e4_pool.tile([128, CQ], BF16, tag="e4")
                    nc.scalar.activation(
                        e4[:].rearrange("p (c q) -> p c q", q=Q),
                        e[:].unsqueeze(-1).broadcast_to((128, C, Q)), AF.Copy,
                        scale=rz[:])
                    # wtt = ih * coef, in place over the broadcast tile
                    nc.vector.scalar_tensor_tensor(
                        e4[:], it, 1.0, e4[:], ALU.bypass, ALU.mult)
                    for h in range(4):
                        sl = slice(512 * h, 512 * (h + 1))
                        nc.tensor.matmul(
                            s_ps[:, sl], sel1_t[:], e4[:, sl],
                            start=first, stop=last_s)
                ar_handles.append(flush_and_allreduce(s_ps, "rb"))

    nc.compile()
    return nc


def _prep_core_inputs(x, W):
    """Host-side shard + repack for one call. Returns list of in_maps."""
    xs_all = np.ascontiguousarray(x)          # [B, I, P]
    in_maps = []
    sel1 = np.tile(np.eye(32, dtype=np.float32), (4, 1)).astype(BF)
    sel32 = (np.tile(np.eye(32, dtype=np.float32), (4, 1)) / C).astype(BF)
    for k in range(N_CORES):
        xs = xs_all[:, k * IL:(k + 1) * IL, :]          # [B, IL, P]
        # lhsT per quad: [128, 256]; even half cols 0:128, odd cols 128:256.
        #   even: col 32j+b (j=0,1) <- xs[b, 4Q+j, p] at partitions 64j+p
        #   odd:  col 64+32j+b      <- xs[b, 4Q+2+j, p] at partitions 64j+p
        xt = xs.transpose(1, 2, 0).reshape(NQUAD, 4, P, B)  # [Q, j, p, b]
        lhsT = np.zeros((NQUAD, 128, 256), np.float32)
        lhsT[:, 0:64, 0:32] = xt[:, 0]
        lhsT[:, 64:128, 32:64] = xt[:, 1]
        lhsT[:, 0:64, 128 + 64:128 + 96] = xt[:, 2]
        lhsT[:, 64:128, 128 + 96:128 + 128] = xt[:, 3]
        lall = np.ascontiguousarray(
            lhsT.astype(BF).transpose(1, 0, 2)).reshape(128, -1)
        Ws = W[k * IL:(k + 1) * IL]                      # [IL, C, P, Q]
        # [p, (c q)] per i, stacked in pairs of two i's on the p axis
        w_rhs = np.ascontiguousarray(
            Ws.reshape(NQUAD, 2, 2, C, P, Q).transpose(0, 1, 2, 4, 3, 5)
        ).reshape(NQUAD, 2, 128, CQ).astype(BF)
        in_maps.append({"lhsT": lall, "w_rhs": np.ascontiguousarray(w_rhs),
                        "sel1": sel1, "sel32": sel32})
    return in_maps


def _host_reference(x, W, bias):
    """Exact numpy fallback (used only if bias != 0, which the problem's
    input spec says cannot happen; the device kernel assumes uniform
    round-0 routing coefficients)."""
    ih = np.einsum("bip,icpq->bicq", x, W)
    b = bias.astype(np.float64)
    out = None
    for r in range(NUM_ROUTING):
        e = np.exp(b - b.max(axis=2, keepdims=True))
        c = e / e.sum(axis=2, keepdims=True)
        s = (c * ih).sum(axis=1, keepdims=True)
        sq = np.sum(s * s, axis=-1, keepdims=True)
        out = s * (sq / (1.0 + sq) / np.sqrt(sq))
        if r != NUM_ROUTING - 1:
            b = b + np.sum(ih * out, axis=-1, keepdims=True)
    return out.reshape(B, C, Q).astype(np.float32)


def kernel(x, W, bias):
    global _compiled
    x = np.asarray(x, dtype=np.float32)
    W = np.asarray(W, dtype=np.float32)
    bias = np.asarray(bias, dtype=np.float32)
    if np.any(bias):
        return _host_reference(x, W, bias)

    if _compiled is None:
        _compiled = _build_kernel()
    nc = _compiled

    in_maps = _prep_core_inputs(x, W)
    res = run_bass_kernel_spmd(
        nc, in_maps, list(range(N_CORES)),
        trace=CONFIG["trace"], trace_cores=CONFIG["trace_cores"],
    )
    kernel.last_results = res
    out = res.results[0]["out"].reshape(B, C, Q)
    return out


# revision 15
# speedup vs baseline: 1.1237x; 1.1237x over previous
"""Trainium2 Bass kernel for CapsuleLayer (dynamic routing) on 8 NeuronCores.

Problem: x[32,1152,64], W[1152,32,64,64], bias[1,1152,32,1] (zeros) ->
         out[32,32,64]
  inputs_hat = einsum('bip,icpq->bicq', x, W)
  3 rounds of routing (softmax over capsule axis, squash, agreement update).

Sharding: input-capsule axis i=1152 split over 8 cores (144 each).
W is read exactly once per core (37.7 MB bf16); only the [32,32,64]
pre-squash sum is AllReduced per routing round (split in two so the first
half overlaps the tail of the round's compute).

v2 layout (vs the HBM-ih baseline):
  - Phase 1 packs FOUR i's per PSUM block: lhsT [128,128] with the even
    i-pair in cols 0:64 and the odd pair in cols 64:128 (zeros elsewhere),
    two matmuls per [128,512] block (start/stop accumulate). Output
    partitions = (islot, b) = 32*islot + b.
  - ih lives in SBUF for the whole kernel ([128, 36*2048] bf16, 144KB per
    partition): no ih HBM write + no 2x reload in the routing rounds.
  - s0 (= sum_i ih / 32, the uniform round-0 weighted sum) via selector
    matmuls (sel/32) over the finished SBUF ih tiles: 4 matmuls of N=512
    per quad, half the PE cost of the baseline's extra-lhsT-column scheme.
  - W tiles stream on two DMA queues (sync + gpsimd issue alternately);
    PSUM evacuation (fp32->bf16) is split Scalar/Vector/Pool.
  - Routing sweeps: scalar_tensor_tensor for the two big [128,2048] muls
    (4x DVE mode), in-place pair-add tree + [128,(32c),16q] reduce for the
    logit dot products, softmax 1/Z folded into the exp-broadcast
    activation's per-partition scale so the selector lhsT is constant
    (one LDWEIGHTS per round), agreement-weighted ih (wtt) computed
    in-place over the exp-broadcast tile.
"""

import os
import sys

import numpy as np

for _p in (
    "/opt/trn_rl_repo",
    "/root/.axon_site",
    "/root/.axon_site/_ro/trn_rl_repo",
    "/root/.axon_site/_ro/pypackages",
):
    if os.path.isdir(_p) and _p not in sys.path:
        sys.path.append(_p)

import ml_dtypes
import concourse.bacc as bacc
import concourse.mybir as mybir
import concourse.tile as tile
from concourse.bass_utils import run_bass_kernel_spmd

F32 = mybir.dt.float32
BF16 = mybir.dt.bfloat16
AF = mybir.ActivationFunctionType
AX = mybir.AxisListType
ALU = mybir.AluOpType
BF = ml_dtypes.bfloat16

B, I, P, C, Q = 32, 1152, 64, 32, 64
N_CORES = 8
IL = I // N_CORES          # 144 input capsules per core
NQUAD = IL // 4            # 36 quads (4 i's x 32 b = 128 partitions)
CQ = C * Q                 # 2048
NUM_ROUTING = 3
SPLIT_QUAD = 26            # s0 partial AllReduced after this many quads
SPLIT_SWEEP = 33           # routing partial AllReduced after this many sweeps

CONFIG = {
    "trace": False,           # profile the run (exec_time_ns); needs ntff hook
    "trace_cores": None,      # None -> core 0 only
}

_compiled = None


def _build_kernel():
    """Build + compile the SPMD Bass module (identical program on 8 cores)."""
    nc = bacc.Bacc("TRN2", target_bir_lowering=False, debug=False,
                   num_devices=N_CORES)

    # lhsT[p, q*256 + (0:128)] = even-pair weights, (128:256) = odd-pair
    lall_d = nc.dram_tensor("lhsT", [128, NQUAD * 256], BF16,
                            kind="ExternalInput")
    # w2[q, 0] = [W(4q);W(4q+1)] stacked on p, [q, 1] = [W(4q+2);W(4q+3)]
    w_d = nc.dram_tensor("w_rhs", [NQUAD, 2, 128, CQ], BF16,
                         kind="ExternalInput")
    sel1_d = nc.dram_tensor("sel1", [128, 32], BF16, kind="ExternalInput")
    sel32_d = nc.dram_tensor("sel32", [128, 32], BF16, kind="ExternalInput")
    out_d = nc.dram_tensor("out", [B, CQ], F32, kind="ExternalOutput")

    rgroups = [list(range(N_CORES))]

    with tile.TileContext(nc) as tc:
        with (
            tc.tile_pool(name="ihp", bufs=1) as ih_pool,
            tc.tile_pool(name="lq", bufs=6) as lq_pool,
            tc.tile_pool(name="w", bufs=4) as w_pool,
            tc.tile_pool(name="pr", bufs=2) as pr_pool,
            tc.tile_pool(name="e4", bufs=2) as e4_pool,
            tc.tile_pool(name="v4", bufs=1) as v4_pool,
            tc.tile_pool(name="small", bufs=3) as small_pool,
            tc.tile_pool(name="acc", bufs=1) as acc_pool,
            tc.tile_pool(name="sv", bufs=1) as sv_pool,
            tc.tile_pool(name="psA", bufs=4, space="PSUM") as psA,
            tc.tile_pool(name="psB", bufs=1, space="PSUM") as psB,
            tc.tile_pool(name="dram", bufs=2, space="DRAM") as dram_pool,
        ):
            sel1_t = small_pool.tile([128, 32], BF16, tag="sel1")
            nc.sync.dma_start(sel1_t[:], sel1_d[:])
            sel32_t = small_pool.tile([128, 32], BF16, tag="sel32")
            nc.sync.dma_start(sel32_t[:], sel32_d[:])

            b_acc = acc_pool.tile([128, NQUAD * 32], F32, tag="bacc")
            nc.vector.memset(b_acc[:], 0.0)

            # ih for the whole core, SBUF-resident: [128=(islot,b), 36*2048]
            ihb = ih_pool.tile([128, NQUAD * CQ], BF16, tag="ihb")

            # Warm-up AllReduce: the first collective pays a large one-time
            # staging/rendezvous cost; burn it here, hidden under phase 1.
            wu_sb = small_pool.tile([32, 16], F32, tag="wu")
            nc.vector.memset(wu_sb[:], 0.0)
            wu_in = dram_pool.tile([32, 16], F32, tag="wu_in")
            wu_out = dram_pool.tile([32, 16], F32, tag="wu_out")
            nc.gpsimd.dma_start(wu_in[:], wu_sb[:])
            nc.gpsimd.collective_compute(
                "AllReduce", ALU.add,
                ins=[wu_in[:].opt()], outs=[wu_out[:].opt()],
                replica_groups=rgroups,
            )

            def flush_and_allreduce(s_ps, tag):
                """PSUM partial -> SBUF -> DRAM -> AllReduce. Returns the
                collective's DRAM output tile (read back as [128, 512])."""
                f_sb = sv_pool.tile([32, CQ], F32, tag="f_sb")
                nc.scalar.copy(f_sb[:], s_ps[:])
                a_in = dram_pool.tile([32, CQ], F32, tag="ar_in")
                a_out = dram_pool.tile([32, CQ], F32, tag="ar_out")
                nc.gpsimd.dma_start(a_in[:], f_sb[:])
                nc.gpsimd.collective_compute(
                    "AllReduce", ALU.add,
                    ins=[a_in[:].opt()], outs=[a_out[:].opt()],
                    replica_groups=rgroups,
                )
                red = sv_pool.tile([128, CQ // 4], F32, tag=tag)
                nc.gpsimd.dma_start(red[:], a_out[:])
                return red

            # ---------------- Phase 1: ih = x @ W, s0 = sum_i ih / 32 -------
            def s0_matmul(q, s_ps, first, last):
                for blk in range(4):
                    sl = slice(512 * blk, 512 * (blk + 1))
                    nc.tensor.matmul(
                        s_ps[:, sl], sel32_t[:],
                        ihb[:, q * CQ + 512 * blk:q * CQ + 512 * (blk + 1)],
                        start=first, stop=last)

            s_ps = psB.tile([32, CQ], F32, tag="sacc")
            ar_handles = []
            evac_engines = [
                lambda o, i: nc.scalar.copy(o, i),
                lambda o, i: nc.vector.tensor_copy(o, i),
                lambda o, i: nc.scalar.copy(o, i),
                lambda o, i: nc.vector.tensor_copy(o, i),
            ]
            for q in range(NQUAD):
                lq = lq_pool.tile([128, 256], BF16, tag="lq")
                nc.sync.dma_start(lq[:], lall_d[:, 256 * q:256 * (q + 1)])
                we = w_pool.tile([128, CQ], BF16, tag="w")
                nc.sync.dma_start(we[:], w_d[q, 0])
                # W-odd on the scalar engine's queue: a third DMA queue, and
                # crucially NOT the gpsimd queue, whose collectives would
                # stall the tail quads' loads.
                wo = w_pool.tile([128, CQ], BF16, tag="w")
                nc.scalar.dma_start(wo[:], w_d[q, 1])
                pss = []
                for blk in range(4):
                    sl = slice(512 * blk, 512 * (blk + 1))
                    ps = psA.tile([128, 512], F32)
                    pss.append(ps)
                    nc.tensor.matmul(ps[:], lq[:, 0:128], we[:, sl],
                                     start=True, stop=False)
                # s0 for the previous quad between the even/odd groups so the
                # PE never waits on the current quad's evacuation
                if q > 0:
                    qc = q - 1
                    s0_matmul(qc, s_ps,
                              first=(qc == 0 or qc == SPLIT_QUAD),
                              last=(qc == SPLIT_QUAD - 1 or qc == NQUAD - 1))
                    if qc == SPLIT_QUAD - 1:
                        ar_handles.append(flush_and_allreduce(s_ps, "ra"))
                        s_ps = psB.tile([32, CQ], F32, tag="sacc")
                for blk in range(4):
                    sl = slice(512 * blk, 512 * (blk + 1))
                    nc.tensor.matmul(pss[blk][:], lq[:, 128:256], wo[:, sl],
                                     start=False, stop=True)
                    evac_engines[blk](
                        ihb[:, q * CQ + 512 * blk:q * CQ + 512 * (blk + 1)],
                        pss[blk][:])
            qc = NQUAD - 1
            s0_matmul(qc, s_ps, first=False, last=True)
            ar_handles.append(flush_and_allreduce(s_ps, "rb"))

            # ---------------- Routing rounds -------------------------------
            # Post-AllReduce S layout: [128, 512], partition p = 4*b + k
            # (k = c-octet), free = (8c, 64q). Squash runs on all 128 lanes.
            C8 = C // 4     # 8 capsules per partition
            for r in range(1, NUM_ROUTING + 1):
                pa, pb = ar_handles
                S_sb = sv_pool.tile([128, CQ // 4], F32, tag="S_sb")
                nc.vector.tensor_add(S_sb[:], pa[:], pb[:])

                # squash: v = S * sqrt(sq)/(1+sq),  sq = sum_q S^2
                S3 = S_sb[:].rearrange("b (c q) -> b c q", q=Q)
                sq = small_pool.tile([128, C8], F32, tag="sq")
                sqr = sv_pool.tile([128, CQ // 4], F32, tag="sqr")
                nc.vector.tensor_mul(sqr[:], S_sb[:], S_sb[:])
                nc.vector.reduce_sum(
                    sq[:], sqr[:].rearrange("b (c q) -> b c q", q=Q),
                    axis=AX.X)
                rt = small_pool.tile([128, C8], F32, tag="rt")
                nc.scalar.sqrt(rt[:], sq[:])
                onep = small_pool.tile([128, C8], F32, tag="onep")
                nc.vector.tensor_scalar_add(onep[:], sq[:], 1.0)
                rden = small_pool.tile([128, C8], F32, tag="rden")
                nc.vector.reciprocal(rden[:], onep[:])
                scale = small_pool.tile([128, C8], F32, tag="scale")
                nc.vector.tensor_mul(scale[:], rt[:], rden[:])
                scale_b = scale[:].unsqueeze(-1).broadcast_to((128, C8, Q))

                if r == NUM_ROUTING:
                    # v overwrites S_sb in place (fp32 output)
                    nc.vector.tensor_mul(S3, S3, scale_b)
                    nc.sync.dma_start(out_d[:], S_sb[:])
                    break

                # v at bf16, then broadcast to the 4 i-slots: v_c's
                # (b, octet)-major partition stream == b-major (c,q) rows,
                # so each v4 slot-group is a straight SBUF->SBUF copy.
                v_c = sv_pool.tile([128, CQ // 4], BF16, tag="v_c")
                nc.vector.tensor_mul(
                    v_c[:].rearrange("b (c q) -> b c q", q=Q), S3, scale_b)
                v4 = v4_pool.tile([128, CQ], BF16, tag="v4")
                for g in range(4):
                    eng = nc.gpsimd if g % 2 == 0 else nc.scalar
                    eng.dma_start(v4[32 * g:32 * (g + 1), :], v_c[:])

                ar_handles = []
                s_ps = psB.tile([32, CQ], F32, tag="sacc")
                for s in range(NQUAD):
                    if s == SPLIT_SWEEP:
                        ar_handles.append(flush_and_allreduce(s_ps, "ra"))
                        s_ps = psB.tile([32, CQ], F32, tag="sacc")
                    first, last_s = (s == 0 or s == SPLIT_SWEEP), \
                        (s == SPLIT_SWEEP - 1 or s == NQUAD - 1)
                    it = ihb[:, s * CQ:(s + 1) * CQ]
                    # logits: dlog[(i,b), c] = sum_q ih*v. DVE tensor_tensor
                    # runs at 2x for packed bf16 (4x needs 8 crossbar inputs;
                    # TRN2 has 7). The otherwise-idle Pool engine takes the
                    # big product on alternating sweeps and the first pair-add
                    # on every 4th to balance the two engines.
                    pr = pr_pool.tile([128, CQ], BF16, tag="pr")
                    pr_eng = nc.gpsimd if s % 2 == 1 else nc.vector
                    pr_eng.tensor_mul(pr[:], it, v4[:])
                    pr3 = pr[:].rearrange("p (c q) -> p c q", q=Q)
                    h1_eng = nc.gpsimd if s % 4 == 2 else nc.vector
                    h1_eng.tensor_add(
                        pr3[:, :, 0:32], pr3[:, :, 0:32], pr3[:, :, 32:64])
                    nc.vector.tensor_add(
                        pr3[:, :, 0:16], pr3[:, :, 0:16], pr3[:, :, 16:32])
                    dlog = small_pool.tile([128, C], F32, tag="dlog")
                    nc.vector.reduce_sum(dlog[:], pr3[:, :, 0:16], axis=AX.X)
                    bsl = b_acc[:, 32 * s:32 * (s + 1)]
                    nc.vector.tensor_add(bsl, bsl, dlog[:])
                    # softmax over c: exp on ACT (Z via accumulator); 1/Z is
                    # folded into the exp->(c,q) broadcast's input scale.
                    e = small_pool.tile([128, C], BF16, tag="e")
                    z = small_pool.tile([128, 1], F32, tag="z")
                    nc.scalar.activation(e[:], bsl, AF.Exp, accum_out=z[:])
                    rz = small_pool.tile([128, 1], F32, tag="rz")
                    nc.vector.reciprocal(rz[:], z[:])
                    e4 = e4_pool.tile([128, CQ], BF16, tag="e4")
                    nc.scalar.activation(
                        e4[:].rearrange("p (c q) -> p c q", q=Q),
                        e[:].unsqueeze(-1).broadcast_to((128, C, Q)), AF.Copy,
                        scale=rz[:])
                    # wtt = ih * coef, in place over the broadcast tile
                    nc.vector.tensor_mul(e4[:], it, e4[:])
                    for h in range(4):
                        sl = slice(512 * h, 512 * (h + 1))
                        nc.tensor.matmul(
                            s_ps[:, sl], sel1_t[:], e4[:, sl],
                            start=first, stop=last_s)
                ar_handles.append(flush_and_allreduce(s_ps, "rb"))

    nc.compile()
    return nc


def _prep_core_inputs(x, W):
    """Host-side shard + repack for one call. Returns list of in_maps."""
    xs_all = np.ascontiguousarray(x)          # [B, I, P]
    in_maps = []
    sel1 = np.tile(np.eye(32, dtype=np.float32), (4, 1)).astype(BF)
    sel32 = (np.tile(np.eye(32, dtype=np.float32), (4, 1)) / C).astype(BF)
    for k in range(N_CORES):
        xs = xs_all[:, k * IL:(k + 1) * IL, :]          # [B, IL, P]
        # lhsT per quad: [128, 256]; even half cols 0:128, odd cols 128:256.
        #   even: col 32j+b (j=0,1) <- xs[b, 4Q+j, p] at partitions 64j+p
        #   odd:  col 64+32j+b      <- xs[b, 4Q+2+j, p] at partitions 64j+p
        xt = xs.transpose(1, 2, 0).reshape(NQUAD, 4, P, B)  # [Q, j, p, b]
        lhsT = np.zeros((NQUAD, 128, 256), np.float32)
        lhsT[:, 0:64, 0:32] = xt[:, 0]
        lhsT[:, 64:128, 32:64] = xt[:, 1]
        lhsT[:, 0:64, 128 + 64:128 + 96] = xt[:, 2]
        lhsT[:, 64:128, 128 + 96:128 + 128] = xt[:, 3]
        lall = np.ascontiguousarray(
            lhsT.astype(BF).transpose(1, 0, 2)).reshape(128, -1)
        Ws = W[k * IL:(k + 1) * IL]                      # [IL, C, P, Q]
        # [p, (c q)] per i, stacked in pairs of two i's on the p axis
        w_rhs = np.ascontiguousarray(
            Ws.reshape(NQUAD, 2, 2, C, P, Q).transpose(0, 1, 2, 4, 3, 5)
        ).reshape(NQUAD, 2, 128, CQ).astype(BF)
        in_maps.append({"lhsT": lall, "w_rhs": np.ascontiguousarray(w_rhs),
                        "sel1": sel1, "sel32": sel32})
    return in_maps


def _host_reference(x, W, bias):
    """Exact numpy fallback (used only if bias != 0, which the problem's
    input spec says cannot happen; the device kernel assumes uniform
    round-0 routing coefficients)."""
    ih = np.einsum("bip,icpq->bicq", x, W)
    b = bias.astype(np.float64)
    out = None
    for r in range(NUM_ROUTING):
        e = np.exp(b - b.max(axis=2, keepdims=True))
        c = e / e.sum(axis=2, keepdims=True)
        s = (c * ih).sum(axis=1, keepdims=True)
        sq = np.sum(s * s, axis=-1, keepdims=True)
        out = s * (sq / (1.0 + sq) / np.sqrt(sq))
        if r != NUM_ROUTING - 1:
            b = b + np.sum(ih * out, axis=-1, keepdims=True)
    return out.reshape(B, C, Q).astype(np.float32)


def kernel(x, W, bias):
    global _compiled
    x = np.asarray(x, dtype=np.float32)
    W = np.asarray(W, dtype=np.float32)
    bias = np.asarray(bias, dtype=np.float32)
    if np.any(bias):
        return _host_reference(x, W, bias)

    if _compiled is None:
        _compiled = _build_kernel()
    nc = _compiled

    in_maps = _prep_core_inputs(x, W)
    res = run_bass_kernel_spmd(
        nc, in_maps, list(range(N_CORES)),
        trace=CONFIG["trace"], trace_cores=CONFIG["trace_cores"],
    )
    kernel.last_results = res
    out = res.results[0]["out"].reshape(B, C, Q)
    return out


# revision 21
# speedup vs baseline: 1.1357x; 1.0107x over previous
"""Trainium2 Bass kernel for CapsuleLayer (dynamic routing) on 8 NeuronCores.

Problem: x[32,1152,64], W[1152,32,64,64], bias[1,1152,32,1] (zeros) ->
         out[32,32,64]
  inputs_hat = einsum('bip,icpq->bicq', x, W)
  3 rounds of routing (softmax over capsule axis, squash, agreement update).

Sharding: input-capsule axis i=1152 split over 8 cores (144 each).
W is read exactly once per core (37.7 MB bf16); only the [32,32,64]
pre-squash sum is AllReduced per routing round (split in two so the first
half overlaps the tail of the round's compute).

v2 layout (vs the HBM-ih baseline):
  - Phase 1 packs FOUR i's per PSUM block: lhsT [128,128] with the even
    i-pair in cols 0:64 and the odd pair in cols 64:128 (zeros elsewhere),
    two matmuls per [128,512] block (start/stop accumulate). Output
    partitions = (islot, b) = 32*islot + b.
  - ih lives in SBUF for the whole kernel ([128, 36*2048] bf16, 144KB per
    partition): no ih HBM write + no 2x reload in the routing rounds.
  - s0 (= sum_i ih / 32, the uniform round-0 weighted sum) via selector
    matmuls (sel/32) over the finished SBUF ih tiles: 4 matmuls of N=512
    per quad, half the PE cost of the baseline's extra-lhsT-column scheme.
  - W tiles stream on two DMA queues (sync + gpsimd issue alternately);
    PSUM evacuation (fp32->bf16) is split Scalar/Vector/Pool.
  - Routing sweeps: scalar_tensor_tensor for the two big [128,2048] muls
    (4x DVE mode), in-place pair-add tree + [128,(32c),16q] reduce for the
    logit dot products, softmax 1/Z folded into the exp-broadcast
    activation's per-partition scale so the selector lhsT is constant
    (one LDWEIGHTS per round), agreement-weighted ih (wtt) computed
    in-place over the exp-broadcast tile.
"""

import os
import sys

import numpy as np

for _p in (
    "/opt/trn_rl_repo",
    "/root/.axon_site",
    "/root/.axon_site/_ro/trn_rl_repo",
    "/root/.axon_site/_ro/pypackages",
):
    if os.path.isdir(_p) and _p not in sys.path:
        sys.path.append(_p)

import ml_dtypes
import concourse.bacc as bacc
import concourse.mybir as mybir
import concourse.tile as tile
from concourse.bass_utils import run_bass_kernel_spmd

F32 = mybir.dt.float32
BF16 = mybir.dt.bfloat16
AF = mybir.ActivationFunctionType
AX = mybir.AxisListType
ALU = mybir.AluOpType
BF = ml_dtypes.bfloat16

B, I, P, C, Q = 32, 1152, 64, 32, 64
N_CORES = 8
IL = I // N_CORES          # 144 input capsules per core
NQUAD = IL // 4            # 36 quads (4 i's x 32 b = 128 partitions)
CQ = C * Q                 # 2048
NUM_ROUTING = 3
SPLIT_QUAD = 26            # s0 partial AllReduced after this many quads
SPLIT_SWEEP = 33           # routing partial AllReduced after this many sweeps

CONFIG = {
    "trace": False,           # profile the run (exec_time_ns); needs ntff hook
    "trace_cores": None,      # None -> core 0 only
}

_compiled = None


def _build_kernel():
    """Build + compile the SPMD Bass module (identical program on 8 cores)."""
    nc = bacc.Bacc("TRN2", target_bir_lowering=False, debug=False,
                   num_devices=N_CORES)

    # lhsT[p, q*256 + (0:128)] = even-pair weights, (128:256) = odd-pair
    lall_d = nc.dram_tensor("lhsT", [128, NQUAD * 256], BF16,
                            kind="ExternalInput")
    # w2[q, 0] = [W(4q);W(4q+1)] stacked on p, [q, 1] = [W(4q+2);W(4q+3)]
    w_d = nc.dram_tensor("w_rhs", [NQUAD, 2, 128, CQ], BF16,
                         kind="ExternalInput")
    sel1_d = nc.dram_tensor("sel1", [128, 32], BF16, kind="ExternalInput")
    sel32_d = nc.dram_tensor("sel32", [128, 32], BF16, kind="ExternalInput")
    out_d = nc.dram_tensor("out", [B, CQ], F32, kind="ExternalOutput")

    rgroups = [list(range(N_CORES))]

    with tile.TileContext(nc) as tc:
        with (
            tc.tile_pool(name="ihp", bufs=1) as ih_pool,
            tc.tile_pool(name="lq", bufs=2) as lq_pool,
            tc.tile_pool(name="w", bufs=4) as w_pool,
            tc.tile_pool(name="pr", bufs=2) as pr_pool,
            tc.tile_pool(name="e4", bufs=1) as e4_pool,
            tc.tile_pool(name="wt", bufs=2) as wt_pool,
            tc.tile_pool(name="v4", bufs=1) as v4_pool,
            tc.tile_pool(name="small", bufs=2) as small_pool,
            tc.tile_pool(name="acc", bufs=1) as acc_pool,
            tc.tile_pool(name="sv", bufs=1) as sv_pool,
            tc.tile_pool(name="psA", bufs=4, space="PSUM") as psA,
            tc.tile_pool(name="psB", bufs=1, space="PSUM") as psB,
            tc.tile_pool(name="dram", bufs=2, space="DRAM") as dram_pool,
        ):
            sel1_t = small_pool.tile([128, 32], BF16, tag="sel1")
            nc.sync.dma_start(sel1_t[:], sel1_d[:])
            sel32_t = small_pool.tile([128, 32], BF16, tag="sel32")
            nc.sync.dma_start(sel32_t[:], sel32_d[:])

            b_acc = acc_pool.tile([128, NQUAD * 32], F32, tag="bacc")
            nc.vector.memset(b_acc[:], 0.0)

            # ih for the whole core, SBUF-resident: [128=(islot,b), 36*2048]
            ihb = ih_pool.tile([128, NQUAD * CQ], BF16, tag="ihb")

            # Warm-up AllReduce: the first collective pays a large one-time
            # staging/rendezvous cost; burn it here, hidden under phase 1.
            wu_sb = small_pool.tile([32, 16], F32, tag="wu")
            nc.vector.memset(wu_sb[:], 0.0)
            wu_in = dram_pool.tile([32, 16], F32, tag="wu_in")
            wu_out = dram_pool.tile([32, 16], F32, tag="wu_out")
            nc.gpsimd.dma_start(wu_in[:], wu_sb[:])
            nc.gpsimd.collective_compute(
                "AllReduce", ALU.add,
                ins=[wu_in[:].opt()], outs=[wu_out[:].opt()],
                replica_groups=rgroups,
            )

            def flush_and_allreduce(s_ps, tag):
                """PSUM partial -> SBUF -> DRAM -> AllReduce. Returns the
                collective's DRAM output tile (read back as [128, 512])."""
                f_sb = sv_pool.tile([32, CQ], F32, tag="f_sb")
                nc.scalar.copy(f_sb[:], s_ps[:])
                a_in = dram_pool.tile([32, CQ], F32, tag="ar_in")
                a_out = dram_pool.tile([32, CQ], F32, tag="ar_out")
                nc.gpsimd.dma_start(a_in[:], f_sb[:])
                nc.gpsimd.collective_compute(
                    "AllReduce", ALU.add,
                    ins=[a_in[:].opt()], outs=[a_out[:].opt()],
                    replica_groups=rgroups,
                )
                red = sv_pool.tile([128, CQ // 4], F32, tag=tag)
                nc.gpsimd.dma_start(red[:], a_out[:])
                return red

            # ---------------- Phase 1: ih = x @ W, s0 = sum_i ih / 32 -------
            def s0_matmul(q, s_ps, first, last):
                for blk in range(4):
                    sl = slice(512 * blk, 512 * (blk + 1))
                    nc.tensor.matmul(
                        s_ps[:, sl], sel32_t[:],
                        ihb[:, q * CQ + 512 * blk:q * CQ + 512 * (blk + 1)],
                        start=first, stop=last)

            s_ps = psB.tile([32, CQ], F32, tag="sacc")
            ar_handles = []
            evac_engines = [
                lambda o, i: nc.scalar.copy(o, i),
                lambda o, i: nc.vector.tensor_copy(o, i),
                lambda o, i: nc.scalar.copy(o, i),
                lambda o, i: nc.vector.tensor_copy(o, i),
            ]
            for q in range(NQUAD):
                lq = lq_pool.tile([128, 256], BF16, tag="lq")
                nc.sync.dma_start(lq[:], lall_d[:, 256 * q:256 * (q + 1)])
                we = w_pool.tile([128, CQ], BF16, tag="w")
                nc.sync.dma_start(we[:], w_d[q, 0])
                # W-odd on the scalar engine's queue: a third DMA queue, and
                # crucially NOT the gpsimd queue, whose collectives would
                # stall the tail quads' loads.
                wo = w_pool.tile([128, CQ], BF16, tag="w")
                nc.scalar.dma_start(wo[:], w_d[q, 1])
                pss = []
                for blk in range(4):
                    sl = slice(512 * blk, 512 * (blk + 1))
                    ps = psA.tile([128, 512], F32)
                    pss.append(ps)
                    nc.tensor.matmul(ps[:], lq[:, 0:128], we[:, sl],
                                     start=True, stop=False)
                # s0 for the previous quad between the even/odd groups so the
                # PE never waits on the current quad's evacuation
                if q > 0:
                    qc = q - 1
                    s0_matmul(qc, s_ps,
                              first=(qc == 0 or qc == SPLIT_QUAD),
                              last=(qc == SPLIT_QUAD - 1 or qc == NQUAD - 1))
                    if qc == SPLIT_QUAD - 1:
                        ar_handles.append(flush_and_allreduce(s_ps, "ra"))
                        s_ps = psB.tile([32, CQ], F32, tag="sacc")
                for blk in range(4):
                    sl = slice(512 * blk, 512 * (blk + 1))
                    nc.tensor.matmul(pss[blk][:], lq[:, 128:256], wo[:, sl],
                                     start=False, stop=True)
                    evac_engines[blk](
                        ihb[:, q * CQ + 512 * blk:q * CQ + 512 * (blk + 1)],
                        pss[blk][:])
            qc = NQUAD - 1
            s0_matmul(qc, s_ps, first=False, last=True)
            ar_handles.append(flush_and_allreduce(s_ps, "rb"))

            # ---------------- Routing rounds -------------------------------
            # Post-AllReduce S layout: [128, 512], partition p = 4*b + k
            # (k = c-octet), free = (8c, 64q). Squash runs on all 128 lanes.
            C8 = C // 4     # 8 capsules per partition
            for r in range(1, NUM_ROUTING + 1):
                pa, pb = ar_handles
                S_sb = sv_pool.tile([128, CQ // 4], F32, tag="S_sb")
                nc.vector.tensor_add(S_sb[:], pa[:], pb[:])

                # squash: v = S * sqrt(sq)/(1+sq),  sq = sum_q S^2
                S3 = S_sb[:].rearrange("b (c q) -> b c q", q=Q)
                sq = small_pool.tile([128, C8], F32, tag="sq")
                sqr = sv_pool.tile([128, CQ // 4], F32, tag="sqr")
                nc.vector.tensor_mul(sqr[:], S_sb[:], S_sb[:])
                nc.vector.reduce_sum(
                    sq[:], sqr[:].rearrange("b (c q) -> b c q", q=Q),
                    axis=AX.X)
                rt = small_pool.tile([128, C8], F32, tag="rt")
                nc.scalar.sqrt(rt[:], sq[:])
                onep = small_pool.tile([128, C8], F32, tag="onep")
                nc.vector.tensor_scalar_add(onep[:], sq[:], 1.0)
                rden = small_pool.tile([128, C8], F32, tag="rden")
                nc.vector.reciprocal(rden[:], onep[:])
                scale = small_pool.tile([128, C8], F32, tag="scale")
                nc.vector.tensor_mul(scale[:], rt[:], rden[:])
                scale_b = scale[:].unsqueeze(-1).broadcast_to((128, C8, Q))

                if r == NUM_ROUTING:
                    # v overwrites S_sb in place (fp32 output)
                    nc.vector.tensor_mul(S3, S3, scale_b)
                    nc.sync.dma_start(out_d[:], S_sb[:])
                    break

                # v at bf16, then broadcast to the 4 i-slots: v_c's
                # (b, octet)-major partition stream == b-major (c,q) rows,
                # so each v4 slot-group is a straight SBUF->SBUF copy.
                v_c = sv_pool.tile([128, CQ // 4], BF16, tag="v_c")
                nc.vector.tensor_mul(
                    v_c[:].rearrange("b (c q) -> b c q", q=Q), S3, scale_b)
                v4 = v4_pool.tile([128, CQ], BF16, tag="v4")
                for g in range(4):
                    eng = nc.gpsimd if g % 2 == 0 else nc.scalar
                    eng.dma_start(v4[32 * g:32 * (g + 1), :], v_c[:])

                ar_handles = []
                s_ps = psB.tile([32, CQ], F32, tag="sacc")
                for s in range(NQUAD):
                    if s == SPLIT_SWEEP:
                        ar_handles.append(flush_and_allreduce(s_ps, "ra"))
                        s_ps = psB.tile([32, CQ], F32, tag="sacc")
                    first, last_s = (s == 0 or s == SPLIT_SWEEP), \
                        (s == SPLIT_SWEEP - 1 or s == NQUAD - 1)
                    it = ihb[:, s * CQ:(s + 1) * CQ]
                    # logits: dlog[(i,b), c] = sum_q ih*v. DVE tensor_tensor
                    # runs at 2x for packed bf16 (4x needs 8 crossbar inputs;
                    # TRN2 has 7). The otherwise-idle Pool engine takes the
                    # big product on alternating sweeps and the first pair-add
                    # on every 4th to balance the two engines.
                    pr = pr_pool.tile([128, CQ], BF16, tag="pr")
                    pr_eng = nc.gpsimd if s % 2 == 1 else nc.vector
                    pr_eng.tensor_mul(pr[:], it, v4[:])
                    pr3 = pr[:].rearrange("p (c q) -> p c q", q=Q)
                    nc.vector.tensor_add(
                        pr3[:, :, 0:32], pr3[:, :, 0:32], pr3[:, :, 32:64])
                    nc.vector.tensor_add(
                        pr3[:, :, 0:16], pr3[:, :, 0:16], pr3[:, :, 16:32])
                    dlog = small_pool.tile([128, C], F32, tag="dlog")
                    nc.vector.reduce_sum(dlog[:], pr3[:, :, 0:16], axis=AX.X)
                    bsl = b_acc[:, 32 * s:32 * (s + 1)]
                    nc.vector.tensor_add(bsl, bsl, dlog[:])
                    # softmax over c: exp on ACT (Z via accumulator); 1/Z is
                    # folded into the exp->(c,q) broadcast's input scale.
                    e = small_pool.tile([128, C], BF16, tag="e")
                    z = small_pool.tile([128, 1], F32, tag="z")
                    nc.scalar.activation(e[:], bsl, AF.Exp, accum_out=z[:])
                    rz = small_pool.tile([128, 1], F32, tag="rz")
                    nc.vector.reciprocal(rz[:], z[:])
                    e4 = e4_pool.tile([128, CQ], BF16, tag="e4")
                    nc.scalar.activation(
                        e4[:].rearrange("p (c q) -> p c q", q=Q),
                        e[:].unsqueeze(-1).broadcast_to((128, C, Q)), AF.Copy,
                        scale=rz[:])
                    # wtt = ih * coef (separate tile: in-place aliasing would
                    # force the DVE's 1x read-modify-write path, ~3.3x slower)
                    wt = wt_pool.tile([128, CQ], BF16, tag="wt")
                    nc.vector.tensor_mul(wt[:], it, e4[:])
                    for h in range(4):
                        sl = slice(512 * h, 512 * (h + 1))
                        nc.tensor.matmul(
                            s_ps[:, sl], sel1_t[:], wt[:, sl],
                            start=first, stop=last_s)
                ar_handles.append(flush_and_allreduce(s_ps, "rb"))

    nc.compile()
    return nc


def _prep_core_inputs(x, W):
    """Host-side shard + repack for one call. Returns list of in_maps."""
    xs_all = np.ascontiguousarray(x)          # [B, I, P]
    in_maps = []
    sel1 = np.tile(np.eye(32, dtype=np.float32), (4, 1)).astype(BF)
    sel32 = (np.tile(np.eye(32, dtype=np.float32), (4, 1)) / C).astype(BF)
    for k in range(N_CORES):
        xs = xs_all[:, k * IL:(k + 1) * IL, :]          # [B, IL, P]
        # lhsT per quad: [128, 256]; even half cols 0:128, odd cols 128:256.
        #   even: col 32j+b (j=0,1) <- xs[b, 4Q+j, p] at partitions 64j+p
        #   odd:  col 64+32j+b      <- xs[b, 4Q+2+j, p] at partitions 64j+p
        xt = xs.transpose(1, 2, 0).reshape(NQUAD, 4, P, B)  # [Q, j, p, b]
        lhsT = np.zeros((NQUAD, 128, 256), np.float32)
        lhsT[:, 0:64, 0:32] = xt[:, 0]
        lhsT[:, 64:128, 32:64] = xt[:, 1]
        lhsT[:, 0:64, 128 + 64:128 + 96] = xt[:, 2]
        lhsT[:, 64:128, 128 + 96:128 + 128] = xt[:, 3]
        lall = np.ascontiguousarray(
            lhsT.astype(BF).transpose(1, 0, 2)).reshape(128, -1)
        Ws = W[k * IL:(k + 1) * IL]                      # [IL, C, P, Q]
        # [p, (c q)] per i, stacked in pairs of two i's on the p axis
        w_rhs = np.ascontiguousarray(
            Ws.reshape(NQUAD, 2, 2, C, P, Q).transpose(0, 1, 2, 4, 3, 5)
        ).reshape(NQUAD, 2, 128, CQ).astype(BF)
        in_maps.append({"lhsT": lall, "w_rhs": np.ascontiguousarray(w_rhs),
                        "sel1": sel1, "sel32": sel32})
    return in_maps


def _host_reference(x, W, bias):
    """Exact numpy fallback (used only if bias != 0, which the problem's
    input spec says cannot happen; the device kernel assumes uniform
    round-0 routing coefficients)."""
    ih = np.einsum("bip,icpq->bicq", x, W)
    b = bias.astype(np.float64)
    out = None
    for r in range(NUM_ROUTING):
        e = np.exp(b - b.max(axis=2, keepdims=True))
        c = e / e.sum(axis=2, keepdims=True)
        s = (c * ih).sum(axis=1, keepdims=True)
        sq = np.sum(s * s, axis=-1, keepdims=True)
        out = s * (sq / (1.0 + sq) / np.sqrt(sq))
        if r != NUM_ROUTING - 1:
            b = b + np.sum(ih * out, axis=-1, keepdims=True)
    return out.reshape(B, C, Q).astype(np.float32)


def kernel(x, W, bias):
    global _compiled
    x = np.asarray(x, dtype=np.float32)
    W = np.asarray(W, dtype=np.float32)
    bias = np.asarray(bias, dtype=np.float32)
    if np.any(bias):
        return _host_reference(x, W, bias)

    if _compiled is None:
        _compiled = _build_kernel()
    nc = _compiled

    in_maps = _prep_core_inputs(x, W)
    res = run_bass_kernel_spmd(
        nc, in_maps, list(range(N_CORES)),
        trace=CONFIG["trace"], trace_cores=CONFIG["trace_cores"],
    )
    kernel.last_results = res
    out = res.results[0]["out"].reshape(B, C, Q)
    return out


# revision 22
# speedup vs baseline: 1.2274x; 1.0808x over previous
"""Trainium2 Bass kernel for CapsuleLayer (dynamic routing) on 8 NeuronCores.

Problem: x[32,1152,64], W[1152,32,64,64], bias[1,1152,32,1] (zeros) ->
         out[32,32,64]
  inputs_hat = einsum('bip,icpq->bicq', x, W)
  3 rounds of routing (softmax over capsule axis, squash, agreement update).

Sharding: input-capsule axis i=1152 split over 8 cores (144 each).
W is read exactly once per core (37.7 MB bf16); only the [32,32,64]
pre-squash sum is AllReduced per routing round (split in two so the first
half overlaps the tail of the round's compute).

v2 layout (vs the HBM-ih baseline):
  - Phase 1 packs FOUR i's per PSUM block: lhsT [128,128] with the even
    i-pair in cols 0:64 and the odd pair in cols 64:128 (zeros elsewhere),
    two matmuls per [128,512] block (start/stop accumulate). Output
    partitions = (islot, b) = 32*islot + b.
  - ih lives in SBUF for the whole kernel ([128, 36*2048] bf16, 144KB per
    partition): no ih HBM write + no 2x reload in the routing rounds.
  - s0 (= sum_i ih / 32, the uniform round-0 weighted sum) via selector
    matmuls (sel/32) over the finished SBUF ih tiles: 4 matmuls of N=512
    per quad, half the PE cost of the baseline's extra-lhsT-column scheme.
  - W tiles stream on two DMA queues (sync + gpsimd issue alternately);
    PSUM evacuation (fp32->bf16) is split Scalar/Vector/Pool.
  - Routing sweeps: scalar_tensor_tensor for the two big [128,2048] muls
    (4x DVE mode), in-place pair-add tree + [128,(32c),16q] reduce for the
    logit dot products, softmax 1/Z folded into the exp-broadcast
    activation's per-partition scale so the selector lhsT is constant
    (one LDWEIGHTS per round), agreement-weighted ih (wtt) computed
    in-place over the exp-broadcast tile.
"""

import os
import sys

import numpy as np

for _p in (
    "/opt/trn_rl_repo",
    "/root/.axon_site",
    "/root/.axon_site/_ro/trn_rl_repo",
    "/root/.axon_site/_ro/pypackages",
):
    if os.path.isdir(_p) and _p not in sys.path:
        sys.path.append(_p)

import ml_dtypes
import concourse.bacc as bacc
import concourse.mybir as mybir
import concourse.tile as tile
from concourse.bass_utils import run_bass_kernel_spmd

F32 = mybir.dt.float32
BF16 = mybir.dt.bfloat16
AF = mybir.ActivationFunctionType
AX = mybir.AxisListType
ALU = mybir.AluOpType
BF = ml_dtypes.bfloat16

B, I, P, C, Q = 32, 1152, 64, 32, 64
N_CORES = 8
IL = I // N_CORES          # 144 input capsules per core
NQUAD = IL // 4            # 36 quads (4 i's x 32 b = 128 partitions)
CQ = C * Q                 # 2048
NUM_ROUTING = 3
SPLIT_QUAD = 26            # s0 partial AllReduced after this many quads
SPLIT_SWEEP = 33           # routing partial AllReduced after this many sweeps

CONFIG = {
    "trace": False,           # profile the run (exec_time_ns); needs ntff hook
    "trace_cores": None,      # None -> core 0 only
}

_compiled = None


def _build_kernel():
    """Build + compile the SPMD Bass module (identical program on 8 cores)."""
    nc = bacc.Bacc("TRN2", target_bir_lowering=False, debug=False,
                   num_devices=N_CORES)

    # lhsT[p, q*256 + (0:128)] = even-pair weights, (128:256) = odd-pair
    lall_d = nc.dram_tensor("lhsT", [128, NQUAD * 256], BF16,
                            kind="ExternalInput")
    # w2[q, 0] = [W(4q);W(4q+1)] stacked on p, [q, 1] = [W(4q+2);W(4q+3)]
    w_d = nc.dram_tensor("w_rhs", [NQUAD, 2, 128, CQ], BF16,
                         kind="ExternalInput")
    sel1_d = nc.dram_tensor("sel1", [128, 32], BF16, kind="ExternalInput")
    sel32_d = nc.dram_tensor("sel32", [128, 32], BF16, kind="ExternalInput")
    out_d = nc.dram_tensor("out", [B, CQ], F32, kind="ExternalOutput")

    rgroups = [list(range(N_CORES))]

    with tile.TileContext(nc) as tc:
        with (
            tc.tile_pool(name="ihp", bufs=1) as ih_pool,
            tc.tile_pool(name="lq", bufs=2) as lq_pool,
            tc.tile_pool(name="w", bufs=4) as w_pool,
            tc.tile_pool(name="pr", bufs=2) as pr_pool,
            tc.tile_pool(name="e4", bufs=1) as e4_pool,
            tc.tile_pool(name="wt", bufs=2) as wt_pool,
            tc.tile_pool(name="v4", bufs=1) as v4_pool,
            tc.tile_pool(name="small", bufs=2) as small_pool,
            tc.tile_pool(name="acc", bufs=1) as acc_pool,
            tc.tile_pool(name="sv", bufs=1) as sv_pool,
            tc.tile_pool(name="psA", bufs=4, space="PSUM") as psA,
            tc.tile_pool(name="psB", bufs=1, space="PSUM") as psB,
            tc.tile_pool(name="dram", bufs=2, space="DRAM") as dram_pool,
        ):
            sel1_t = small_pool.tile([128, 32], BF16, tag="sel1")
            nc.sync.dma_start(sel1_t[:], sel1_d[:])
            sel32_t = small_pool.tile([128, 32], BF16, tag="sel32")
            nc.sync.dma_start(sel32_t[:], sel32_d[:])

            b_acc = acc_pool.tile([128, NQUAD * 32], F32, tag="bacc")
            nc.vector.memset(b_acc[:], 0.0)

            # ih for the whole core, SBUF-resident: [128=(islot,b), 36*2048]
            ihb = ih_pool.tile([128, NQUAD * CQ], BF16, tag="ihb")

            # Warm-up AllReduce: the first collective pays a large one-time
            # staging/rendezvous cost; burn it here, hidden under phase 1.
            wu_sb = small_pool.tile([32, 16], F32, tag="wu")
            nc.vector.memset(wu_sb[:], 0.0)
            wu_in = dram_pool.tile([32, 16], F32, tag="wu_in")
            wu_out = dram_pool.tile([32, 16], F32, tag="wu_out")
            nc.gpsimd.dma_start(wu_in[:], wu_sb[:])
            nc.gpsimd.collective_compute(
                "AllReduce", ALU.add,
                ins=[wu_in[:].opt()], outs=[wu_out[:].opt()],
                replica_groups=rgroups,
            )

            def flush_and_allreduce(s_ps, tag):
                """PSUM partial -> SBUF -> DRAM -> AllReduce. Returns the
                collective's DRAM output tile (read back as [128, 512])."""
                f_sb = sv_pool.tile([32, CQ], F32, tag="f_sb")
                nc.scalar.copy(f_sb[:], s_ps[:])
                a_in = dram_pool.tile([32, CQ], F32, tag="ar_in")
                a_out = dram_pool.tile([32, CQ], F32, tag="ar_out")
                nc.gpsimd.dma_start(a_in[:], f_sb[:])
                nc.gpsimd.collective_compute(
                    "AllReduce", ALU.add,
                    ins=[a_in[:].opt()], outs=[a_out[:].opt()],
                    replica_groups=rgroups,
                )
                red = sv_pool.tile([128, CQ // 4], F32, tag=tag)
                nc.gpsimd.dma_start(red[:], a_out[:])
                return red

            # ---------------- Phase 1: ih = x @ W, s0 = sum_i ih / 32 -------
            def s0_matmul(q, s_ps, first, last):
                for blk in range(4):
                    sl = slice(512 * blk, 512 * (blk + 1))
                    nc.tensor.matmul(
                        s_ps[:, sl], sel32_t[:],
                        ihb[:, q * CQ + 512 * blk:q * CQ + 512 * (blk + 1)],
                        start=first, stop=last)

            s_ps = psB.tile([32, CQ], F32, tag="sacc")
            ar_handles = []
            evac_engines = [
                lambda o, i: nc.scalar.copy(o, i),
                lambda o, i: nc.vector.tensor_copy(o, i),
                lambda o, i: nc.scalar.copy(o, i),
                lambda o, i: nc.vector.tensor_copy(o, i),
            ]
            for q in range(NQUAD):
                lq = lq_pool.tile([128, 256], BF16, tag="lq")
                nc.sync.dma_start(lq[:], lall_d[:, 256 * q:256 * (q + 1)])
                we = w_pool.tile([128, CQ], BF16, tag="w")
                nc.sync.dma_start(we[:], w_d[q, 0])
                # W-odd on the scalar engine's queue: a third DMA queue, and
                # crucially NOT the gpsimd queue, whose collectives would
                # stall the tail quads' loads.
                wo = w_pool.tile([128, CQ], BF16, tag="w")
                nc.scalar.dma_start(wo[:], w_d[q, 1])
                pss = []
                for blk in range(4):
                    sl = slice(512 * blk, 512 * (blk + 1))
                    ps = psA.tile([128, 512], F32)
                    pss.append(ps)
                    nc.tensor.matmul(ps[:], lq[:, 0:128], we[:, sl],
                                     start=True, stop=False)
                # s0 for the previous quad between the even/odd groups so the
                # PE never waits on the current quad's evacuation
                if q > 0:
                    qc = q - 1
                    s0_matmul(qc, s_ps,
                              first=(qc == 0 or qc == SPLIT_QUAD),
                              last=(qc == SPLIT_QUAD - 1 or qc == NQUAD - 1))
                    if qc == SPLIT_QUAD - 1:
                        ar_handles.append(flush_and_allreduce(s_ps, "ra"))
                        s_ps = psB.tile([32, CQ], F32, tag="sacc")
                for blk in range(4):
                    sl = slice(512 * blk, 512 * (blk + 1))
                    nc.tensor.matmul(pss[blk][:], lq[:, 128:256], wo[:, sl],
                                     start=False, stop=True)
                    evac_engines[blk](
                        ihb[:, q * CQ + 512 * blk:q * CQ + 512 * (blk + 1)],
                        pss[blk][:])
            qc = NQUAD - 1
            s0_matmul(qc, s_ps, first=False, last=True)
            ar_handles.append(flush_and_allreduce(s_ps, "rb"))

            # ---------------- Routing rounds -------------------------------
            # Post-AllReduce S layout: [128, 512], partition p = 4*b + k
            # (k = c-octet), free = (8c, 64q). Squash runs on all 128 lanes.
            C8 = C // 4     # 8 capsules per partition
            for r in range(1, NUM_ROUTING + 1):
                pa, pb = ar_handles
                S_sb = sv_pool.tile([128, CQ // 4], F32, tag="S_sb")
                nc.vector.tensor_add(S_sb[:], pa[:], pb[:])

                # squash: v = S * sqrt(sq)/(1+sq),  sq = sum_q S^2
                S3 = S_sb[:].rearrange("b (c q) -> b c q", q=Q)
                sq = small_pool.tile([128, C8], F32, tag="sq")
                sqr = sv_pool.tile([128, CQ // 4], F32, tag="sqr")
                nc.vector.tensor_mul(sqr[:], S_sb[:], S_sb[:])
                nc.vector.reduce_sum(
                    sq[:], sqr[:].rearrange("b (c q) -> b c q", q=Q),
                    axis=AX.X)
                rt = small_pool.tile([128, C8], F32, tag="rt")
                nc.scalar.sqrt(rt[:], sq[:])
                onep = small_pool.tile([128, C8], F32, tag="onep")
                nc.vector.tensor_scalar_add(onep[:], sq[:], 1.0)
                rden = small_pool.tile([128, C8], F32, tag="rden")
                nc.vector.reciprocal(rden[:], onep[:])
                scale = small_pool.tile([128, C8], F32, tag="scale")
                nc.vector.tensor_mul(scale[:], rt[:], rden[:])
                scale_b = scale[:].unsqueeze(-1).broadcast_to((128, C8, Q))

                if r == NUM_ROUTING:
                    # v overwrites S_sb in place (fp32 output)
                    nc.vector.tensor_mul(S3, S3, scale_b)
                    nc.sync.dma_start(out_d[:], S_sb[:])
                    break

                # v at bf16, then broadcast to the 4 i-slots: v_c's
                # (b, octet)-major partition stream == b-major (c,q) rows,
                # so each v4 slot-group is a straight SBUF->SBUF copy.
                v_c = sv_pool.tile([128, CQ // 4], BF16, tag="v_c")
                nc.vector.tensor_mul(
                    v_c[:].rearrange("b (c q) -> b c q", q=Q), S3, scale_b)
                v4 = v4_pool.tile([128, CQ], BF16, tag="v4")
                for g in range(4):
                    eng = nc.gpsimd if g % 2 == 0 else nc.scalar
                    eng.dma_start(v4[32 * g:32 * (g + 1), :], v_c[:])

                ar_handles = []
                s_ps = psB.tile([32, CQ], F32, tag="sacc")
                for s in range(NQUAD):
                    if s == SPLIT_SWEEP:
                        ar_handles.append(flush_and_allreduce(s_ps, "ra"))
                        s_ps = psB.tile([32, CQ], F32, tag="sacc")
                    first, last_s = (s == 0 or s == SPLIT_SWEEP), \
                        (s == SPLIT_SWEEP - 1 or s == NQUAD - 1)
                    it = ihb[:, s * CQ:(s + 1) * CQ]
                    # logits: dlog[(i,b), c] = sum_q ih*v. DVE tensor_tensor
                    # runs at 2x for packed bf16 (4x needs 8 crossbar inputs;
                    # TRN2 has 7). No Pool-engine offload: a concurrent
                    # GpSimd stream degrades DVE throughput ~3x (SBUF port
                    # contention), a net loss while DVE is the bottleneck.
                    pr = pr_pool.tile([128, CQ], BF16, tag="pr")
                    nc.vector.tensor_mul(pr[:], it, v4[:])
                    pr3 = pr[:].rearrange("p (c q) -> p c q", q=Q)
                    # bf16 reduce keeps every operand 2-byte, enabling the
                    # DVE 2x path; the logits only steer routing, so the
                    # rounding is well inside tolerance.
                    dlog = small_pool.tile([128, C], BF16, tag="dlog")
                    with nc.allow_low_precision("routing logits"):
                        nc.vector.reduce_sum(dlog[:], pr3, axis=AX.X)
                    bsl = b_acc[:, 32 * s:32 * (s + 1)]
                    nc.vector.tensor_add(bsl, bsl, dlog[:])
                    # softmax over c: exp on ACT (Z via accumulator); 1/Z is
                    # folded into the exp->(c,q) broadcast's input scale.
                    e = small_pool.tile([128, C], BF16, tag="e")
                    z = small_pool.tile([128, 1], F32, tag="z")
                    nc.scalar.activation(e[:], bsl, AF.Exp, accum_out=z[:])
                    rz = small_pool.tile([128, 1], F32, tag="rz")
                    nc.vector.reciprocal(rz[:], z[:])
                    e4 = e4_pool.tile([128, CQ], BF16, tag="e4")
                    nc.scalar.activation(
                        e4[:].rearrange("p (c q) -> p c q", q=Q),
                        e[:].unsqueeze(-1).broadcast_to((128, C, Q)), AF.Copy,
                        scale=rz[:])
                    # wtt = ih * coef (separate tile: in-place aliasing would
                    # force the DVE's 1x read-modify-write path, ~3.3x slower)
                    wt = wt_pool.tile([128, CQ], BF16, tag="wt")
                    nc.vector.tensor_mul(wt[:], it, e4[:])
                    for h in range(4):
                        sl = slice(512 * h, 512 * (h + 1))
                        nc.tensor.matmul(
                            s_ps[:, sl], sel1_t[:], wt[:, sl],
                            start=first, stop=last_s)
                ar_handles.append(flush_and_allreduce(s_ps, "rb"))

    nc.compile()
    return nc


def _prep_core_inputs(x, W):
    """Host-side shard + repack for one call. Returns list of in_maps."""
    xs_all = np.ascontiguousarray(x)          # [B, I, P]
    in_maps = []
    sel1 = np.tile(np.eye(32, dtype=np.float32), (4, 1)).astype(BF)
    sel32 = (np.tile(np.eye(32, dtype=np.float32), (4, 1)) / C).astype(BF)
    for k in range(N_CORES):
        xs = xs_all[:, k * IL:(k + 1) * IL, :]          # [B, IL, P]
        # lhsT per quad: [128, 256]; even half cols 0:128, odd cols 128:256.
        #   even: col 32j+b (j=0,1) <- xs[b, 4Q+j, p] at partitions 64j+p
        #   odd:  col 64+32j+b      <- xs[b, 4Q+2+j, p] at partitions 64j+p
        xt = xs.transpose(1, 2, 0).reshape(NQUAD, 4, P, B)  # [Q, j, p, b]
        lhsT = np.zeros((NQUAD, 128, 256), np.float32)
        lhsT[:, 0:64, 0:32] = xt[:, 0]
        lhsT[:, 64:128, 32:64] = xt[:, 1]
        lhsT[:, 0:64, 128 + 64:128 + 96] = xt[:, 2]
        lhsT[:, 64:128, 128 + 96:128 + 128] = xt[:, 3]
        lall = np.ascontiguousarray(
            lhsT.astype(BF).transpose(1, 0, 2)).reshape(128, -1)
        Ws = W[k * IL:(k + 1) * IL]                      # [IL, C, P, Q]
        # [p, (c q)] per i, stacked in pairs of two i's on the p axis
        w_rhs = np.ascontiguousarray(
            Ws.reshape(NQUAD, 2, 2, C, P, Q).transpose(0, 1, 2, 4, 3, 5)
        ).reshape(NQUAD, 2, 128, CQ).astype(BF)
        in_maps.append({"lhsT": lall, "w_rhs": np.ascontiguousarray(w_rhs),
                        "sel1": sel1, "sel32": sel32})
    return in_maps


def _host_reference(x, W, bias):
    """Exact numpy fallback (used only if bias != 0, which the problem's
    input spec says cannot happen; the device kernel assumes uniform
    round-0 routing coefficients)."""
    ih = np.einsum("bip,icpq->bicq", x, W)
    b = bias.astype(np.float64)
    out = None
    for r in range(NUM_ROUTING):
        e = np.exp(b - b.max(axis=2, keepdims=True))
        c = e / e.sum(axis=2, keepdims=True)
        s = (c * ih).sum(axis=1, keepdims=True)
        sq = np.sum(s * s, axis=-1, keepdims=True)
        out = s * (sq / (1.0 + sq) / np.sqrt(sq))
        if r != NUM_ROUTING - 1:
            b = b + np.sum(ih * out, axis=-1, keepdims=True)
    return out.reshape(B, C, Q).astype(np.float32)


def kernel(x, W, bias):
    global _compiled
    x = np.asarray(x, dtype=np.float32)
    W = np.asarray(W, dtype=np.float32)
    bias = np.asarray(bias, dtype=np.float32)
    if np.any(bias):
        return _host_reference(x, W, bias)

    if _compiled is None:
        _compiled = _build_kernel()
    nc = _compiled

    in_maps = _prep_core_inputs(x, W)
    res = run_bass_kernel_spmd(
        nc, in_maps, list(range(N_CORES)),
        trace=CONFIG["trace"], trace_cores=CONFIG["trace_cores"],
    )
    kernel.last_results = res
    out = res.results[0]["out"].reshape(B, C, Q)
    return out
